# revision 31
# baseline (speedup 1.0000x reference)
"""CrossAttention on 8 TRN2 NeuronCores (tensor-parallel over heads).

Reference computation (B=4, N=2048, DIM=1024, 16 heads, head_dim=64):
    qkv = x @ Wqkv.T + bqkv ; q, k = split(qkv)  (v unused)
    attn = softmax(q @ k.T * scale) ; out = attn @ split_heads(context)
    return merge_heads(out) @ Wout.T + bout

Sharding: core c owns heads {2c, 2c+1}. Each core computes q/k
projections for its heads (full sequence), head-parallel attention with
context slices as values, then a per-batch AllToAll re-shards from
head-parallel to row-parallel so the output projection runs locally.
Row ownership is interleaved (core c owns rows [c*256:(c+1)*256] of
every batch).

Steady state is ScalarE-bound (exp of the full attention matrix, one
[128,1024] ACTIVATE per key-chunk). The emission interleaves the q/k
projection of batch b+1 and the output projection of batch b-1 as small
matmul units inside attention(b)'s key-chunk loop so neither TensorE
nor ScalarE ever starves and the PE stays HAM-warm to the end.
"""
import numpy as np
import ml_dtypes

import concourse.bass as bass
import concourse.mybir as mybir
import concourse.tile as tile
from concourse import bacc
from concourse.bass_utils import run_bass_kernel_spmd

BF16 = ml_dtypes.bfloat16
F32 = mybir.dt.float32
BF = mybir.dt.bfloat16

NC = 8            # cores
B = 4             # batch
N = 2048          # sequence
DIM = 1024
NH = 16           # heads total
HD = 64           # head dim
HPC = NH // NC    # heads per core = 2
SCALE = HD ** -0.5
BN = B * N        # 8192 tokens
RPB = N // NC     # rows per (core, batch) after re-shard = 256
KC = DIM // 128   # contraction chunks for projections = 8
NKC = N // 128    # key chunks per batch = 16
CW = HD + 1       # value width incl. ones column = 65
NT = 4            # 512-token chunks per batch
PT_BUFS = 8


def build():
    nc = bacc.Bacc("TRN2", target_bir_lowering=False, debug=False,
                   num_devices=NC)

    xT = nc.dram_tensor("xT", [DIM, BN], BF, kind="ExternalInput")
    wqkT = nc.dram_tensor("wqkT", [DIM, 2 * 128], BF, kind="ExternalInput")
    bqk = nc.dram_tensor("bqk", [2 * 128, 1], F32, kind="ExternalInput")
    ctxa = nc.dram_tensor("ctxa", [B, HPC, 128, NKC * CW], BF,
                          kind="ExternalInput")
    woutT = nc.dram_tensor("woutT", [DIM, DIM], BF, kind="ExternalInput")
    boutb = nc.dram_tensor("boutb", [128, DIM], F32, kind="ExternalInput")
    # out rows: batch-major, 256 rows per batch (this core's rows)
    out = nc.dram_tensor("out", [B * RPB, DIM], F32, kind="ExternalOutput")

    # per-(batch, half) AllToAll bounce buffers; in half hf, chunk j
    # holds rows [hf*1024 + j*128 : hf*1024 + (j+1)*128] of batch b and
    # is delivered to core j. Core c therefore owns two 128-row stripes
    # per batch: hf*1024 + c*128 for hf in {0, 1}.
    a2a_in = [[nc.dram_tensor(f"a2a_in{b}_{hf}", [NC, 128, 128], BF)
               for hf in range(2)] for b in range(B)]
    a2a_out = [[nc.dram_tensor(f"a2a_out{b}_{hf}", [NC, 128, 128], BF)
                for hf in range(2)] for b in range(B)]

    rscr = [nc.dram_tensor(f"rscr{i}", [1, 512], F32) for i in range(16)]
    _scr_idx = [0]



    with tile.TileContext(nc) as tc:
        with tc.tile_pool(name="const", bufs=1) as const, \
             tc.tile_pool(name="qk", bufs=1) as qkpool, \
             tc.tile_pool(name="xt", bufs=40) as xtpool, \
             tc.tile_pool(name="pt", bufs=PT_BUFS) as ptpool, \
             tc.tile_pool(name="r1", bufs=4) as r1pool, \
             tc.tile_pool(name="rb", bufs=4) as rbpool, \
             tc.tile_pool(name="ho", bufs=4) as hopool, \
             tc.tile_pool(name="sl", bufs=16) as slpool, \
             tc.tile_pool(name="ob", bufs=4) as obpool, \
             tc.tile_pool(name="pc", bufs=4) as pcpool, \
             tc.tile_pool(name="pss", bufs=2, space="PSUM") as pss_pool, \
             tc.tile_pool(name="pav", bufs=2, space="PSUM") as pav_pool, \
             tc.tile_pool(name="pj", bufs=2, space="PSUM") as pj_pool:

            # ---- small constants needed up front ----
            wqk_sb = []
            for kc in range(KC):
                t = const.tile([128, 256], BF, tag=f"wqk{kc}")
                nc.sync.dma_start(out=t[:], in_=wqkT[kc * 128:(kc + 1) * 128, :])
                wqk_sb.append(t)
            bq_sb = []
            for fb in range(2):
                t = const.tile([128, 1], F32, tag=f"bq{fb}")
                nc.sync.dma_start(out=t[:], in_=bqk[fb * 128:(fb + 1) * 128, :])
                bq_sb.append(t)
            # warm the exp activation-table set before it is on the
            # critical path (ACT_TABLE_LOAD is ~2.7us)
            tl = r1pool.tile([128, 1], F32, tag="tblwarm", name="tblwarm")
            nc.scalar.activation(tl[:], bq_sb[0][:],
                                 mybir.ActivationFunctionType.Exp)

            wout_sb = []
            bout_sb = const.tile([128, DIM], F32, tag="bout")
            ctx_sb = {}
            qk_tiles = {}
            xt_tiles = {}

            def warm_pe(n, name):
                """Dummy matmuls to hold the HAM clock gate at K=8/8
                across a PE-idle window (content is irrelevant)."""
                wp = pav_pool.tile([128, 256], F32, tag="pav",
                                   name=f"warm_{name}")
                for i in range(n):
                    nc.tensor.matmul(wp[:], wqk_sb[0][:, 0:128],
                                     wqk_sb[0][:], start=True, stop=True)

            def load_out_consts():
                for fc in range(KC):
                    t = const.tile([128, DIM], BF, tag=f"wout{fc}",
                                   name=f"wout{fc}")
                    nc.sync.dma_start(
                        out=t[:], in_=woutT[fc * 128:(fc + 1) * 128, :])
                    wout_sb.append(t)
                nc.sync.dma_start(out=bout_sb[:], in_=boutb[:])

            def load_ctx(b):
                for h in range(HPC):
                    t = const.tile([128, NKC * CW], BF, tag=f"ctx{b % 2}{h}",
                                   name=f"ctx{b}_{h}")
                    nc.sync.dma_start(out=t[:], in_=ctxa[b, h, :, :])
                    ctx_sb[b, h] = t

            def prefetch_x(b):
                """Issue per-(kc, t) xT DMAs and allocate q/k for batch b."""
                qT = qkpool.tile([128, N], BF, tag=f"qT{b % 2}", name=f"qT{b}")
                kT = qkpool.tile([128, N], BF, tag=f"kT{b % 2}", name=f"kT{b}")
                qk_tiles[b] = (qT, kT)
                for t in range(NT):
                    for kc in range(KC):
                        xt = xtpool.tile([128, 512], BF, tag="xt",
                                         name=f"xtb{b}_{kc}_{t}")
                        nc.sync.dma_start(
                            out=xt[:],
                            in_=xT[kc * 128:(kc + 1) * 128,
                                   b * N + t * 512:b * N + (t + 1) * 512])
                        xt_tiles[b, kc, t] = xt

            def qkproj_unit(b, t, fb):
                """Project one (512-token, q-or-k) slice of batch b."""
                qT, kT = qk_tiles[b]
                dst = kT if fb == 1 else qT
                ps = pj_pool.tile([128, 512], F32, tag="pj",
                                  name=f"psq{b}_{t}_{fb}")
                for kc in range(KC):
                    nc.tensor.matmul(
                        ps[:], wqk_sb[kc][:, fb * 128:(fb + 1) * 128],
                        xt_tiles[b, kc, t][:],
                        start=(kc == 0), stop=(kc == KC - 1))
                nc.vector.tensor_scalar_add(
                    dst[:, t * 512:(t + 1) * 512], ps[:], bq_sb[fb][:])

            def outproj_unit(b, rc, n):
                """512 output features (n) for row-stripe rc of batch b."""
                if n == 0:
                    sls = []
                    for fc in range(KC):
                        sl = slpool.tile([128, 128], BF, tag="sl",
                                         name=f"sl{b}_{rc}_{fc}")
                        nc.sync.dma_start(
                            out=sl[:], in_=a2a_out[b][rc][fc, :, :])
                        sls.append(sl)
                    outproj_unit.sls[b, rc] = sls
                sls = outproj_unit.sls[b, rc]
                pso = pj_pool.tile([128, 512], F32, tag="pj",
                                   name=f"pso{b}_{rc}_{n}")
                for fc in range(KC):
                    nc.tensor.matmul(
                        pso[:], sls[fc][:],
                        wout_sb[fc][:, n * 512:(n + 1) * 512],
                        start=(fc == 0), stop=(fc == KC - 1))
                ob = obpool.tile([128, 512], F32, tag="ob",
                                 name=f"ob{b}_{rc}_{n}")
                nc.vector.tensor_tensor(
                    out=ob[:], in0=pso[:],
                    in1=bout_sb[:, n * 512:(n + 1) * 512],
                    op=mybir.AluOpType.add)
                nc.sync.dma_start(
                    out=out[b * RPB + rc * 128:b * RPB + (rc + 1) * 128,
                            n * 512:(n + 1) * 512],
                    in_=ob[:])
                # out row b*256 + rc*128 + i holds batch-b global row
                # rc*1024 + c*128 + i (stripe ownership)
            outproj_unit.sls = {}

            def attention_qg(b, qg, fillers):
                """Both heads' scores+softmax+values for 512 queries.

                fillers: dict kc -> list of thunks emitted after that
                key-chunk's exp (projection work woven into the stream).
                """
                qT, kT = qk_tiles[b]
                q0 = qg * 512
                pts = []
                for kc in range(NKC):
                    ps = pss_pool.tile([128, 1024], F32, tag="pss",
                                       name=f"pss{b}{qg}{kc}")
                    for h in range(HPC):
                        nc.tensor.matmul(
                            ps[:, h * 512:(h + 1) * 512],
                            kT[h * HD:(h + 1) * HD, kc * 128:(kc + 1) * 128],
                            qT[h * HD:(h + 1) * HD, q0:q0 + 512],
                            start=True, stop=True,
                            tile_position=(h * HD, 0))
                    pt = ptpool.tile([128, 1024], BF, tag="pt",
                                     name=f"pt{b}_{qg}_{kc}")
                    nc.scalar.activation(
                        pt[:], ps[:],
                        mybir.ActivationFunctionType.Exp, scale=SCALE)
                    pts.append(pt)
                    for f in fillers.get(kc, ()):
                        f()
                # stage-major normalize: PSUM evictions first (frees pav
                # slots for the next group's chains), then reciprocals.
                # The returned closure finishes later — an on-chip K=1
                # matmul broadcasts 1/den across the 64 head-dim
                # partitions (no DRAM round-trip) — woven into the NEXT
                # group's stream so the slow reciprocal never stalls PE.
                pcs, r1s = [], []
                for h in range(HPC):
                    pav = pav_pool.tile([CW, 512], F32, tag="pav",
                                        name=f"pav{b}{qg}{h}")
                    for kc in range(NKC):
                        nc.tensor.matmul(
                            pav[:], ctx_sb[b, h][:, kc * CW:(kc + 1) * CW],
                            pts[kc][:, h * 512:(h + 1) * 512],
                            start=(kc == 0), stop=(kc == NKC - 1))
                    pc = pcpool.tile([CW, 512], F32, tag="pc",
                                     name=f"pc{b}{qg}{h}")
                    nc.vector.tensor_copy(pc[:], pav[:])
                    pcs.append(pc)
                for h in range(HPC):
                    r1 = r1pool.tile([1, 512], F32, tag="r1",
                                     name=f"r1{b}{qg}{h}")
                    nc.vector.reciprocal(r1[:], pcs[h][HD:CW, :])
                    r1s.append(r1)

                # broadcast 1/den to 64 partitions via a DRAM round-trip
                # (pure DMA — never touches the PE stream); issue now so
                # the bounce is in flight before finish() consumes it
                rbs = []
                for h in range(HPC):
                    scr = rscr[_scr_idx[0] % 16]; _scr_idx[0] += 1
                    nc.sync.dma_start(out=scr[:], in_=r1s[h][:])
                    rb = rbpool.tile([HD, 512], F32, tag="rb",
                                     name=f"rb{b}{qg}{h}")
                    nc.sync.dma_start(out=rb[:],
                                      in_=scr[:].broadcast_to([HD, 512]))
                    rbs.append(rb)

                def finish(b=b, qg=qg, q0=q0, pcs=pcs, rbs=rbs):
                    for h in range(HPC):
                        ho = hopool.tile([HD, 512], BF, tag="ho",
                                         name=f"ho{b}{qg}{h}")
                        nc.vector.tensor_tensor(
                            out=ho[:], in0=pcs[h][0:HD, :], in1=rbs[h][:],
                            op=mybir.AluOpType.mult)
                        for m in range(4):
                            row = q0 + m * 128
                            hf, j = row // 1024, (row % 1024) // 128
                            nc.sync.dma_start(
                                out=a2a_in[b][hf][j, h * HD:(h + 1) * HD, :],
                                in_=ho[:, m * 128:(m + 1) * 128])
                return finish

            def reshard(b, hf):
                nc.gpsimd.collective_compute(
                    "AllToAll", mybir.AluOpType.bypass,
                    replica_groups=[list(range(NC))],
                    ins=[a2a_in[b][hf].ap().opt()],
                    outs=[a2a_out[b][hf].ap().opt()])

            # ---------------- emission ----------------
            prefetch_x(0)
            load_ctx(0)
            # warm the PE clock while the first x chunks are in flight
            warm_pe(40, "head")
            # scores(qg0, kc) only needs k of token-chunk kc//4 and q of
            # chunk 0 — project those, then start attention immediately
            # and weave the remaining projection slices into qg0/qg1
            qkproj_unit(0, 0, 1)
            qkproj_unit(0, 0, 0)
            prefetch_x(1)
            load_ctx(1)
            load_out_consts()

            pending = []  # deferred normalize-finish of the previous group
            for b in range(B):
                for qg in range(4):
                    fillers = {}
                    if pending:
                        fillers.setdefault(6, []).append(pending.pop())
                    if b == 0:
                        # remaining b0 slices: k(t) must land before
                        # scores reach kc = 4*t; q(t) before group t
                        b0_slots = {0: ((0, 1, 1), (4, 2, 1), (8, 3, 1),
                                        (12, 1, 0)),
                                    1: ((4, 2, 0), (12, 3, 0))}
                        for slot, t, fb in b0_slots.get(qg, ()):
                            fillers.setdefault(slot, []).append(
                                lambda t=t, fb=fb: qkproj_unit(0, t, fb))
                    if b + 1 < B:
                        # q/k projection of the next batch: 8 units per
                        # batch (kept out of b0's warm-up query group)
                        if b == 0:
                            b1_slots = {1: ((2, 0, 1), (10, 0, 0)),
                                        2: ((1, 1, 1), (5, 2, 1),
                                            (9, 1, 0), (13, 2, 0)),
                                        3: ((1, 3, 1), (9, 3, 0))}
                            for slot, t, fb in b1_slots.get(qg, ()):
                                fillers.setdefault(slot, []).append(
                                    lambda t=t, fb=fb: qkproj_unit(1, t, fb))
                        else:
                            t = qg
                            fillers.setdefault(1, []).append(
                                lambda b=b, t=t: qkproj_unit(b + 1, t, 1))
                            fillers.setdefault(9, []).append(
                                lambda b=b, t=t: qkproj_unit(b + 1, t, 0))
                    if b >= 1 and qg >= 2:
                        # output projection of the previous batch in the
                        # second half of this batch's attention, so its
                        # AllToAll has certainly landed
                        rc = qg - 2
                        fillers.setdefault(5, []).append(
                            lambda b=b, rc=rc: outproj_unit(b - 1, rc, 0))
                        fillers.setdefault(13, []).append(
                            lambda b=b, rc=rc: outproj_unit(b - 1, rc, 1))
                    fin = attention_qg(b, qg, fillers)
                    if qg == 3:
                        # last group of the batch: finish inline so the
                        # second-half collective can be emitted now (its
                        # dependency set must include these DMAs)
                        fin()
                    else:
                        pending.append(fin)
                    if qg == 2:
                        # both first-half finishes (qg0 in qg1, qg1 here)
                        # have been emitted by now
                        reshard(b, 0)
                        if b + 2 < B:
                            prefetch_x(b + 2)
                            load_ctx(b + 2)
                reshard(b, 1)
            # tail: hold the PE warm across the final normalize window,
            # run stripe-0's output projection (its a2a landed mid-batch)
            # while the last collective flies, then stripe 1
            warm_pe(80, "tail0")
            for n in range(2):
                outproj_unit(3, 0, n)
            warm_pe(48, "tail1")
            for n in range(2):
                outproj_unit(3, 1, n)
    nc.compile()
    return nc


def prep_inputs(x, context, Wqkv, bqkv, Wout, bout):
    """Host-side sharding: returns in_maps for the 8 cores."""
    x = np.asarray(x, np.float32)
    context = np.asarray(context, np.float32)
    Wqkv = np.asarray(Wqkv, np.float32)
    bqkv = np.asarray(bqkv, np.float32)
    Wout = np.asarray(Wout, np.float32)
    bout = np.asarray(bout, np.float32)

    xT = np.ascontiguousarray(x.reshape(BN, DIM).T).astype(BF16)
    woutT = np.ascontiguousarray(Wout.T).astype(BF16)
    boutb = np.broadcast_to(bout, (128, DIM)).astype(np.float32).copy()

    in_maps = []
    for c in range(NC):
        h0 = c * HPC
        # feature order: [q_h0 | q_h1] then [k_h0 | k_h1]
        wq = Wqkv[h0 * HD:(h0 + HPC) * HD]
        wk = Wqkv[DIM + h0 * HD:DIM + (h0 + HPC) * HD]
        wqkT = np.ascontiguousarray(
            np.concatenate([wq, wk], axis=0).T).astype(BF16)
        bq = np.concatenate([bqkv[h0 * HD:(h0 + HPC) * HD],
                             bqkv[DIM + h0 * HD:DIM + (h0 + HPC) * HD]])
        bq = bq.reshape(2 * 128, 1).astype(np.float32)
        ctxa = np.ones((B, HPC, 128, NKC, CW), np.float32)
        for h in range(HPC):
            g = h0 + h
            arr = context[:, :, g * HD:(g + 1) * HD].reshape(B, NKC, 128, HD)
            ctxa[:, h, :, :, :HD] = arr.transpose(0, 2, 1, 3)
        in_maps.append({
            "xT": xT,
            "wqkT": wqkT,
            "bqk": bq,
            "ctxa": ctxa.reshape(B, HPC, 128, NKC * CW).astype(BF16),
            "woutT": woutT,
            "boutb": boutb,
        })
    return in_maps


_NC_CACHE = None


def _get_nc():
    global _NC_CACHE
    if _NC_CACHE is None:
        _NC_CACHE = build()
    return _NC_CACHE


def run(in_maps, trace=False):
    nc = _get_nc()
    res = run_bass_kernel_spmd(nc, in_maps, core_ids=list(range(NC)),
                               trace=trace)
    # core c's out = [B, 2, 128, DIM]: stripe (b, hf) holds batch-b rows
    # [hf*1024 + c*128 : hf*1024 + (c+1)*128]
    full = np.empty((B, N, DIM), np.float32)
    for c in range(NC):
        o = np.asarray(res.results[c]["out"]).reshape(B, 2, 128, DIM)
        for hf in range(2):
            full[:, hf * 1024 + c * 128:hf * 1024 + (c + 1) * 128, :] = \
                o[:, hf]
    return full, res


def kernel(x, context, Wqkv, bqkv, Wout, bout):
    in_maps = prep_inputs(x, context, Wqkv, bqkv, Wout, bout)
    out, _ = run(in_maps, trace=False)
    return out


# revision 39
# speedup vs baseline: 1.0328x; 1.0328x over previous
"""CrossAttention on 8 TRN2 NeuronCores (tensor-parallel over heads).

Reference computation (B=4, N=2048, DIM=1024, 16 heads, head_dim=64):
    qkv = x @ Wqkv.T + bqkv ; q, k = split(qkv)  (v unused)
    attn = softmax(q @ k.T * scale) ; out = attn @ split_heads(context)
    return merge_heads(out) @ Wout.T + bout

Sharding: core c owns heads {2c, 2c+1}. Each core computes q/k
projections for its heads (full sequence), head-parallel attention with
context slices as values, then a per-batch AllToAll re-shards from
head-parallel to row-parallel so the output projection runs locally.
Row ownership is interleaved (core c owns rows [c*256:(c+1)*256] of
every batch).

Steady state is ScalarE-bound (exp of the full attention matrix, one
[128,1024] ACTIVATE per key-chunk). The emission interleaves the q/k
projection of batch b+1 and the output projection of batch b-1 as small
matmul units inside attention(b)'s key-chunk loop so neither TensorE
nor ScalarE ever starves and the PE stays HAM-warm to the end.
"""
import numpy as np
import ml_dtypes

import concourse.bass as bass
import concourse.mybir as mybir
import concourse.tile as tile
from concourse import bacc
from concourse.bass_utils import run_bass_kernel_spmd

BF16 = ml_dtypes.bfloat16
F32 = mybir.dt.float32
BF = mybir.dt.bfloat16

NC = 8            # cores
B = 4             # batch
N = 2048          # sequence
DIM = 1024
NH = 16           # heads total
HD = 64           # head dim
HPC = NH // NC    # heads per core = 2
SCALE = HD ** -0.5
BN = B * N        # 8192 tokens
RPB = N // NC     # rows per (core, batch) after re-shard = 256
KC = DIM // 128   # contraction chunks for projections = 8
NKC = N // 128    # key chunks per batch = 16
CW = HD + 1       # value width incl. ones column = 65
NT = 4            # 512-token chunks per batch
PT_BUFS = 8


def build():
    nc = bacc.Bacc("TRN2", target_bir_lowering=False, debug=False,
                   num_devices=NC)

    xT = nc.dram_tensor("xT", [DIM, BN], BF, kind="ExternalInput")
    wqkT = nc.dram_tensor("wqkT", [DIM, 2 * 128], BF, kind="ExternalInput")
    bqk = nc.dram_tensor("bqk", [2 * 128, 1], F32, kind="ExternalInput")
    ctxa = nc.dram_tensor("ctxa", [B, HPC, 128, NKC * CW], BF,
                          kind="ExternalInput")
    woutT = nc.dram_tensor("woutT", [DIM, DIM], BF, kind="ExternalInput")
    boutb = nc.dram_tensor("boutb", [128, DIM], F32, kind="ExternalInput")
    # out rows: batch-major, 256 rows per batch (this core's rows)
    out = nc.dram_tensor("out", [B * RPB, DIM], F32, kind="ExternalOutput")

    # AllToAll bounce buffers. Batches 0-2: one collective per batch,
    # chunk j holds rows [j*256:(j+1)*256] (core c owns rows c*256..).
    # Batch 3 is split in two half-batch collectives (stripe layout,
    # chunk j of half hf = rows hf*1024 + j*128) so the final reshard
    # covers only the last two query groups and the tail stays short.
    a2a_in = [nc.dram_tensor(f"a2a_in{b}", [NC, 128, RPB], BF)
              for b in range(3)]
    a2a_out = [nc.dram_tensor(f"a2a_out{b}", [NC, 128, RPB], BF)
               for b in range(3)]
    a2a3_in = [nc.dram_tensor(f"a2a3_in{hf}", [NC, 128, 128], BF)
               for hf in range(2)]
    a2a3_out = [nc.dram_tensor(f"a2a3_out{hf}", [NC, 128, 128], BF)
                for hf in range(2)]

    rscr = [nc.dram_tensor(f"rscr{i}", [1, 1024], F32) for i in range(8)]
    _scr_idx = [0]



    with tile.TileContext(nc) as tc:
        with tc.tile_pool(name="const", bufs=1) as const, \
             tc.tile_pool(name="qk", bufs=1) as qkpool, \
             tc.tile_pool(name="xt", bufs=40) as xtpool, \
             tc.tile_pool(name="pt", bufs=PT_BUFS) as ptpool, \
             tc.tile_pool(name="r1", bufs=4) as r1pool, \
             tc.tile_pool(name="rb", bufs=4) as rbpool, \
             tc.tile_pool(name="ho", bufs=4) as hopool, \
             tc.tile_pool(name="sl", bufs=16) as slpool, \
             tc.tile_pool(name="ob", bufs=4) as obpool, \
             tc.tile_pool(name="pc", bufs=4) as pcpool, \
             tc.tile_pool(name="pss", bufs=2, space="PSUM") as pss_pool, \
             tc.tile_pool(name="pav", bufs=2, space="PSUM") as pav_pool, \
             tc.tile_pool(name="pj", bufs=2, space="PSUM") as pj_pool:

            # ---- small constants needed up front ----
            wqk_sb = []
            for kc in range(KC):
                t = const.tile([128, 256], BF, tag=f"wqk{kc}")
                nc.sync.dma_start(out=t[:], in_=wqkT[kc * 128:(kc + 1) * 128, :])
                wqk_sb.append(t)
            bq_sb = []
            for fb in range(2):
                t = const.tile([128, 1], F32, tag=f"bq{fb}")
                nc.sync.dma_start(out=t[:], in_=bqk[fb * 128:(fb + 1) * 128, :])
                bq_sb.append(t)
            # warm the exp activation-table set before it is on the
            # critical path (ACT_TABLE_LOAD is ~2.7us)
            tl = r1pool.tile([128, 1], F32, tag="tblwarm", name="tblwarm")
            nc.scalar.activation(tl[:], bq_sb[0][:],
                                 mybir.ActivationFunctionType.Exp)

            wout_sb = []
            bout_sb = const.tile([128, DIM], F32, tag="bout")
            ctx_sb = {}
            qk_tiles = {}
            xt_tiles = {}

            def warm_pe(n, name):
                """Dummy matmuls to hold the HAM clock gate at K=8/8
                across a PE-idle window (content is irrelevant)."""
                wp = pav_pool.tile([128, 256], F32, tag="pav",
                                   name=f"warm_{name}")
                for i in range(n):
                    nc.tensor.matmul(wp[:], wqk_sb[0][:, 0:128],
                                     wqk_sb[0][:], start=True, stop=True)

            def load_out_consts():
                for fc in range(KC):
                    t = const.tile([128, DIM], BF, tag=f"wout{fc}",
                                   name=f"wout{fc}")
                    nc.sync.dma_start(
                        out=t[:], in_=woutT[fc * 128:(fc + 1) * 128, :])
                    wout_sb.append(t)
                nc.sync.dma_start(out=bout_sb[:], in_=boutb[:])

            def load_ctx(b):
                for h in range(HPC):
                    t = const.tile([128, NKC * CW], BF, tag=f"ctx{b % 2}{h}",
                                   name=f"ctx{b}_{h}")
                    nc.sync.dma_start(out=t[:], in_=ctxa[b, h, :, :])
                    ctx_sb[b, h] = t

            def prefetch_x(b):
                """Issue per-(kc, t) xT DMAs and allocate q/k for batch b."""
                qT = qkpool.tile([128, N], BF, tag=f"qT{b % 2}", name=f"qT{b}")
                kT = qkpool.tile([128, N], BF, tag=f"kT{b % 2}", name=f"kT{b}")
                qk_tiles[b] = (qT, kT)
                for t in range(NT):
                    for kc in range(KC):
                        xt = xtpool.tile([128, 512], BF, tag="xt",
                                         name=f"xtb{b}_{kc}_{t}")
                        nc.sync.dma_start(
                            out=xt[:],
                            in_=xT[kc * 128:(kc + 1) * 128,
                                   b * N + t * 512:b * N + (t + 1) * 512])
                        xt_tiles[b, kc, t] = xt

            def qkproj_unit(b, t, fb):
                """Project one (512-token, q-or-k) slice of batch b."""
                qT, kT = qk_tiles[b]
                dst = kT if fb == 1 else qT
                ps = pj_pool.tile([128, 512], F32, tag="pj",
                                  name=f"psq{b}_{t}_{fb}")
                for kc in range(KC):
                    nc.tensor.matmul(
                        ps[:], wqk_sb[kc][:, fb * 128:(fb + 1) * 128],
                        xt_tiles[b, kc, t][:],
                        start=(kc == 0), stop=(kc == KC - 1))
                nc.vector.tensor_scalar_add(
                    dst[:, t * 512:(t + 1) * 512], ps[:], bq_sb[fb][:])

            def outproj_load(b, rc):
                """Stage the re-sharded activations for (b, row-chunk rc).

                For b<3 chunk rc covers out rows b*256+rc*128 (global
                rows c*256+rc*128); for b=3, stripe rc covers global
                rows rc*1024+c*128.
                """
                sls = []
                for fc in range(KC):
                    sl = slpool.tile([128, 128], BF, tag="sl",
                                     name=f"sl{b}_{rc}_{fc}")
                    if b < 3:
                        src = a2a_out[b][fc, :, rc * 128:(rc + 1) * 128]
                    else:
                        src = a2a3_out[rc][fc, :, :]
                    nc.sync.dma_start(out=sl[:], in_=src)
                    sls.append(sl)
                outproj_load.sls[b, rc] = sls
            outproj_load.sls = {}

            def outproj_unit(b, rc, n):
                """512 output features (n) for row-chunk rc of batch b."""
                sls = outproj_load.sls[b, rc]
                pso = pj_pool.tile([128, 512], F32, tag="pj",
                                   name=f"pso{b}_{rc}_{n}")
                for fc in range(KC):
                    nc.tensor.matmul(
                        pso[:], sls[fc][:],
                        wout_sb[fc][:, n * 512:(n + 1) * 512],
                        start=(fc == 0), stop=(fc == KC - 1))
                ob = obpool.tile([128, 512], F32, tag="ob",
                                 name=f"ob{b}_{rc}_{n}")
                nc.vector.tensor_tensor(
                    out=ob[:], in0=pso[:],
                    in1=bout_sb[:, n * 512:(n + 1) * 512],
                    op=mybir.AluOpType.add)
                nc.sync.dma_start(
                    out=out[b * RPB + rc * 128:b * RPB + (rc + 1) * 128,
                            n * 512:(n + 1) * 512],
                    in_=ob[:])

            def attention_qg(b, qg, fillers):
                """Both heads' scores+softmax+values for 512 queries.

                fillers: dict kc -> list of thunks emitted after that
                key-chunk's exp (projection work woven into the stream).
                """
                qT, kT = qk_tiles[b]
                q0 = qg * 512
                pts = []
                for kc in range(NKC):
                    ps = pss_pool.tile([128, 1024], F32, tag="pss",
                                       name=f"pss{b}{qg}{kc}")
                    for h in range(HPC):
                        nc.tensor.matmul(
                            ps[:, h * 512:(h + 1) * 512],
                            kT[h * HD:(h + 1) * HD, kc * 128:(kc + 1) * 128],
                            qT[h * HD:(h + 1) * HD, q0:q0 + 512],
                            start=True, stop=True,
                            tile_position=(h * HD, 0))
                    pt = ptpool.tile([128, 1024], BF, tag="pt",
                                     name=f"pt{b}_{qg}_{kc}")
                    nc.scalar.activation(
                        pt[:], ps[:],
                        mybir.ActivationFunctionType.Exp, scale=SCALE)
                    pts.append(pt)
                    for f in fillers.get(kc, ()):
                        f()
                # stage-major normalize: PSUM evictions now (frees pav
                # slots for the next group's chains); the reciprocal and
                # the DRAM-round-trip broadcast (finish_a) and the
                # normalize-multiply + re-shard staging (finish_b) are
                # woven into the NEXT group's stream so the slow
                # reciprocal never gates this pipeline.
                pcs = []
                for h in range(HPC):
                    pav = pav_pool.tile([CW, 512], F32, tag="pav",
                                        name=f"pav{b}{qg}{h}")
                    for kc in range(NKC):
                        nc.tensor.matmul(
                            pav[:], ctx_sb[b, h][:, kc * CW:(kc + 1) * CW],
                            pts[kc][:, h * 512:(h + 1) * 512],
                            start=(kc == 0), stop=(kc == NKC - 1))
                    pc = pcpool.tile([CW, 512], F32, tag="pc",
                                     name=f"pc{b}{qg}{h}")
                    nc.vector.tensor_copy(pc[:], pav[:])
                    pcs.append(pc)

                rb = rbpool.tile([HD, 1024], F32, tag="rb",
                                 name=f"rb{b}{qg}")

                def finish_a(b=b, qg=qg, pcs=pcs, rb=rb):
                    r1 = r1pool.tile([1, 1024], F32, tag="r1",
                                     name=f"r1{b}{qg}")
                    for h in range(HPC):
                        nc.vector.reciprocal(r1[:, h * 512:(h + 1) * 512],
                                             pcs[h][HD:CW, :])
                    scr = rscr[_scr_idx[0] % 8]; _scr_idx[0] += 1
                    nc.sync.dma_start(out=scr[:], in_=r1[:])
                    nc.sync.dma_start(out=rb[:],
                                      in_=scr[:].broadcast_to([HD, 1024]))

                def finish_b(b=b, qg=qg, q0=q0, pcs=pcs, rb=rb):
                    for h in range(HPC):
                        ho = hopool.tile([HD, 512], BF, tag="ho",
                                         name=f"ho{b}{qg}{h}")
                        nc.vector.tensor_tensor(
                            out=ho[:], in0=pcs[h][0:HD, :],
                            in1=rb[:, h * 512:(h + 1) * 512],
                            op=mybir.AluOpType.mult)
                        if b < 3:
                            for half in range(2):
                                j = 2 * qg + half
                                nc.sync.dma_start(
                                    out=a2a_in[b][j, h * HD:(h + 1) * HD, :],
                                    in_=ho[:, half * 256:(half + 1) * 256])
                        else:
                            for m in range(4):
                                row = q0 + m * 128
                                hf, j = row // 1024, (row % 1024) // 128
                                nc.sync.dma_start(
                                    out=a2a3_in[hf][j, h * HD:(h + 1) * HD,
                                                    :],
                                    in_=ho[:, m * 128:(m + 1) * 128])
                return finish_a, finish_b

            def reshard(b):
                nc.gpsimd.collective_compute(
                    "AllToAll", mybir.AluOpType.bypass,
                    replica_groups=[list(range(NC))],
                    ins=[a2a_in[b].ap().opt()],
                    outs=[a2a_out[b].ap().opt()])

            def reshard3(hf):
                nc.gpsimd.collective_compute(
                    "AllToAll", mybir.AluOpType.bypass,
                    replica_groups=[list(range(NC))],
                    ins=[a2a3_in[hf].ap().opt()],
                    outs=[a2a3_out[hf].ap().opt()])

            # ---------------- emission ----------------
            prefetch_x(0)
            load_ctx(0)
            # warm the PE clock while the first x chunks are in flight
            warm_pe(40, "head")
            # scores(qg0, kc) only needs k of token-chunk kc//4 and q of
            # chunk 0 — project those, then start attention immediately
            # and weave the remaining projection slices into qg0/qg1
            qkproj_unit(0, 0, 1)
            qkproj_unit(0, 0, 0)
            prefetch_x(1)
            load_ctx(1)
            load_out_consts()

            pending = []  # deferred normalize-finish of the previous group
            for b in range(B):
                for qg in range(4):
                    fillers = {}
                    if pending:
                        fa, fb_ = pending.pop()
                        fillers.setdefault(2, []).append(fa)
                        fillers.setdefault(12, []).append(fb_)
                    if b == 0:
                        # remaining b0 slices: k(t) must land before
                        # scores reach kc = 4*t; q(t) before group t
                        b0_slots = {0: ((0, 1, 1), (4, 2, 1), (8, 3, 1),
                                        (12, 1, 0)),
                                    1: ((4, 2, 0), (12, 3, 0))}
                        for slot, t, fb in b0_slots.get(qg, ()):
                            fillers.setdefault(slot, []).append(
                                lambda t=t, fb=fb: qkproj_unit(0, t, fb))
                    if b + 1 < B:
                        # q/k projection of the next batch: 8 units per
                        # batch (kept out of b0's warm-up query group)
                        if b == 0:
                            b1_slots = {1: ((3, 0, 1), (10, 0, 0)),
                                        2: ((1, 1, 1), (6, 2, 1),
                                            (10, 1, 0), (14, 2, 0)),
                                        3: ((1, 3, 1), (10, 3, 0))}
                            for slot, t, fb in b1_slots.get(qg, ()):
                                fillers.setdefault(slot, []).append(
                                    lambda t=t, fb=fb: qkproj_unit(1, t, fb))
                        else:
                            t = qg
                            fillers.setdefault(1, []).append(
                                lambda b=b, t=t: qkproj_unit(b + 1, t, 1))
                            fillers.setdefault(11, []).append(
                                lambda b=b, t=t: qkproj_unit(b + 1, t, 0))
                    if b >= 1 and qg >= 2:
                        # output projection of the previous batch in the
                        # second half of this batch's attention, so its
                        # AllToAll has certainly landed; the loads go
                        # first so the matmuls never wait on DMA
                        rc = qg - 2
                        fillers.setdefault(0, []).append(
                            lambda b=b, rc=rc: outproj_load(b - 1, rc))
                        fillers.setdefault(5, []).append(
                            lambda b=b, rc=rc: outproj_unit(b - 1, rc, 0))
                        fillers.setdefault(13, []).append(
                            lambda b=b, rc=rc: outproj_unit(b - 1, rc, 1))
                    fins = attention_qg(b, qg, fillers)
                    if qg == 3:
                        # last group of the batch: finish inline so the
                        # batch collective can be emitted right after
                        # (its dependency set must include these DMAs)
                        fins[0]()
                        fins[1]()
                    else:
                        pending.append(fins)
                    if b == 3 and qg == 2:
                        # first-half finishes of b3 (qg0 in qg1, qg1
                        # here) have been emitted by now
                        reshard3(0)
                    if qg == 2 and b + 2 < B:
                        prefetch_x(b + 2)
                        load_ctx(b + 2)
                if b < 3:
                    reshard(b)
                else:
                    reshard3(1)
            # tail: hold the PE warm across the final normalize window,
            # run stripe-0's output projection (its a2a landed after
            # qg2) while the last collective flies, then stripe 1
            warm_pe(80, "tail0")
            outproj_load(3, 0)
            for n in range(2):
                outproj_unit(3, 0, n)
            warm_pe(24, "tail1")
            outproj_load(3, 1)
            for n in range(2):
                outproj_unit(3, 1, n)
    nc.compile()
    return nc


def prep_inputs(x, context, Wqkv, bqkv, Wout, bout):
    """Host-side sharding: returns in_maps for the 8 cores."""
    x = np.asarray(x, np.float32)
    context = np.asarray(context, np.float32)
    Wqkv = np.asarray(Wqkv, np.float32)
    bqkv = np.asarray(bqkv, np.float32)
    Wout = np.asarray(Wout, np.float32)
    bout = np.asarray(bout, np.float32)

    xT = np.ascontiguousarray(x.reshape(BN, DIM).T).astype(BF16)
    woutT = np.ascontiguousarray(Wout.T).astype(BF16)
    boutb = np.broadcast_to(bout, (128, DIM)).astype(np.float32).copy()

    in_maps = []
    for c in range(NC):
        h0 = c * HPC
        # feature order: [q_h0 | q_h1] then [k_h0 | k_h1]
        wq = Wqkv[h0 * HD:(h0 + HPC) * HD]
        wk = Wqkv[DIM + h0 * HD:DIM + (h0 + HPC) * HD]
        wqkT = np.ascontiguousarray(
            np.concatenate([wq, wk], axis=0).T).astype(BF16)
        bq = np.concatenate([bqkv[h0 * HD:(h0 + HPC) * HD],
                             bqkv[DIM + h0 * HD:DIM + (h0 + HPC) * HD]])
        bq = bq.reshape(2 * 128, 1).astype(np.float32)
        ctxa = np.ones((B, HPC, 128, NKC, CW), np.float32)
        for h in range(HPC):
            g = h0 + h
            arr = context[:, :, g * HD:(g + 1) * HD].reshape(B, NKC, 128, HD)
            ctxa[:, h, :, :, :HD] = arr.transpose(0, 2, 1, 3)
        in_maps.append({
            "xT": xT,
            "wqkT": wqkT,
            "bqk": bq,
            "ctxa": ctxa.reshape(B, HPC, 128, NKC * CW).astype(BF16),
            "woutT": woutT,
            "boutb": boutb,
        })
    return in_maps


_NC_CACHE = None


def _get_nc():
    global _NC_CACHE
    if _NC_CACHE is None:
        _NC_CACHE = build()
    return _NC_CACHE


def run(in_maps, trace=False):
    nc = _get_nc()
    res = run_bass_kernel_spmd(nc, in_maps, core_ids=list(range(NC)),
                               trace=trace)
    # core c's out = [B*256, DIM]: batches 0-2 hold rows
    # [c*256:(c+1)*256]; batch 3's two 128-row chunks hold stripes
    # [hf*1024 + c*128 : hf*1024 + (c+1)*128]
    full = np.empty((B, N, DIM), np.float32)
    for c in range(NC):
        o = np.asarray(res.results[c]["out"]).reshape(B, RPB, DIM)
        full[:3, c * RPB:(c + 1) * RPB, :] = o[:3]
        for hf in range(2):
            full[3, hf * 1024 + c * 128:hf * 1024 + (c + 1) * 128, :] = \
                o[3, hf * 128:(hf + 1) * 128]
    return full, res


def kernel(x, context, Wqkv, bqkv, Wout, bout):
    in_maps = prep_inputs(x, context, Wqkv, bqkv, Wout, bout)
    out, _ = run(in_maps, trace=False)
    return out


# revision 43
# speedup vs baseline: 1.1070x; 1.0719x over previous
"""CrossAttention on 8 TRN2 NeuronCores (tensor-parallel over heads).

Reference computation (B=4, N=2048, DIM=1024, 16 heads, head_dim=64):
    qkv = x @ Wqkv.T + bqkv ; q, k = split(qkv)  (v unused)
    attn = softmax(q @ k.T * scale) ; out = attn @ split_heads(context)
    return merge_heads(out) @ Wout.T + bout

Sharding: core c owns heads {2c, 2c+1}. Each core computes q/k
projections for its heads (full sequence), head-parallel attention with
context slices as values, then a per-batch AllToAll re-shards from
head-parallel to row-parallel so the output projection runs locally.
Row ownership is interleaved (core c owns rows [c*256:(c+1)*256] of
every batch).

Steady state is ScalarE-bound (exp of the full attention matrix, one
[128,1024] ACTIVATE per key-chunk). The emission interleaves the q/k
projection of batch b+1 and the output projection of batch b-1 as small
matmul units inside attention(b)'s key-chunk loop so neither TensorE
nor ScalarE ever starves and the PE stays HAM-warm to the end.
"""
import numpy as np
import ml_dtypes

import concourse.bass as bass
import concourse.mybir as mybir
import concourse.tile as tile
from concourse import bacc
from concourse.bass_utils import run_bass_kernel_spmd

BF16 = ml_dtypes.bfloat16
F32 = mybir.dt.float32
BF = mybir.dt.bfloat16

NC = 8            # cores
B = 4             # batch
N = 2048          # sequence
DIM = 1024
NH = 16           # heads total
HD = 64           # head dim
HPC = NH // NC    # heads per core = 2
SCALE = HD ** -0.5
BN = B * N        # 8192 tokens
RPB = N // NC     # rows per (core, batch) after re-shard = 256
KC = DIM // 128   # contraction chunks for projections = 8
NKC = N // 128    # key chunks per batch = 16
CW = HD + 1       # value width incl. ones column = 65
NT = 4            # 512-token chunks per batch
PT_BUFS = 8


def build():
    nc = bacc.Bacc("TRN2", target_bir_lowering=False, debug=False,
                   num_devices=NC)

    xT = nc.dram_tensor("xT", [DIM, BN], BF, kind="ExternalInput")
    wqkT = nc.dram_tensor("wqkT", [DIM, 2 * 128], BF, kind="ExternalInput")
    bqk = nc.dram_tensor("bqk", [2 * 128, 1], F32, kind="ExternalInput")
    ctxa = nc.dram_tensor("ctxa", [B, HPC, 128, NKC * CW], BF,
                          kind="ExternalInput")
    woutT = nc.dram_tensor("woutT", [DIM, DIM], BF, kind="ExternalInput")
    boutb = nc.dram_tensor("boutb", [128, DIM], F32, kind="ExternalInput")
    # out rows: batch-major, 256 rows per batch (this core's rows)
    out = nc.dram_tensor("out", [B * RPB, DIM], F32, kind="ExternalOutput")

    # AllToAll bounce buffers. Batches 0-2: one collective per batch,
    # chunk j holds rows [j*256:(j+1)*256] (core c owns rows c*256..).
    # Batch 3 is split in two half-batch collectives (stripe layout,
    # chunk j of half hf = rows hf*1024 + j*128) so the final reshard
    # covers only the last two query groups and the tail stays short.
    a2a_in = [nc.dram_tensor(f"a2a_in{b}", [NC, 128, RPB], BF)
              for b in range(3)]
    a2a_out = [nc.dram_tensor(f"a2a_out{b}", [NC, 128, RPB], BF)
               for b in range(3)]
    a2a3_in = [nc.dram_tensor(f"a2a3_in{hf}", [NC, 128, 128], BF)
               for hf in range(2)]
    a2a3_out = [nc.dram_tensor(f"a2a3_out{hf}", [NC, 128, 128], BF)
                for hf in range(2)]

    rscr = [nc.dram_tensor(f"rscr{i}", [1, 1024], F32) for i in range(8)]
    _scr_idx = [0]



    with tile.TileContext(nc) as tc:
        with tc.tile_pool(name="const", bufs=1) as const, \
             tc.tile_pool(name="qk", bufs=1) as qkpool, \
             tc.tile_pool(name="xt", bufs=40) as xtpool, \
             tc.tile_pool(name="pt", bufs=PT_BUFS) as ptpool, \
             tc.tile_pool(name="r1", bufs=4) as r1pool, \
             tc.tile_pool(name="rb", bufs=4) as rbpool, \
             tc.tile_pool(name="ho", bufs=4) as hopool, \
             tc.tile_pool(name="sl", bufs=16) as slpool, \
             tc.tile_pool(name="ob", bufs=4) as obpool, \
             tc.tile_pool(name="pc", bufs=4) as pcpool, \
             tc.tile_pool(name="pss", bufs=2, space="PSUM") as pss_pool, \
             tc.tile_pool(name="pav", bufs=2, space="PSUM") as pav_pool, \
             tc.tile_pool(name="pj", bufs=2, space="PSUM") as pj_pool:

            # ---- small constants needed up front ----
            wqk_sb = []
            for kc in range(KC):
                t = const.tile([128, 256], BF, tag=f"wqk{kc}")
                nc.sync.dma_start(out=t[:], in_=wqkT[kc * 128:(kc + 1) * 128, :])
                wqk_sb.append(t)
            bq_sb = []
            for fb in range(2):
                t = const.tile([128, 1], F32, tag=f"bq{fb}")
                nc.sync.dma_start(out=t[:], in_=bqk[fb * 128:(fb + 1) * 128, :])
                bq_sb.append(t)
            # warm the exp activation-table set before it is on the
            # critical path (ACT_TABLE_LOAD is ~2.7us)
            tl = r1pool.tile([128, 1], F32, tag="tblwarm", name="tblwarm")
            nc.scalar.activation(tl[:], bq_sb[0][:],
                                 mybir.ActivationFunctionType.Exp)

            wout_sb = []
            bout_sb = const.tile([128, DIM], F32, tag="bout")
            ctx_sb = {}
            qk_tiles = {}
            xt_tiles = {}

            def warm_pe(n, name):
                """Dummy matmuls to hold the HAM clock gate at K=8/8
                across a PE-idle window (content is irrelevant)."""
                wp = pav_pool.tile([128, 256], F32, tag="pav",
                                   name=f"warm_{name}")
                for i in range(n):
                    nc.tensor.matmul(wp[:], wqk_sb[0][:, 0:128],
                                     wqk_sb[0][:], start=True, stop=True)

            def load_out_consts():
                for fc in range(KC):
                    t = const.tile([128, DIM], BF, tag=f"wout{fc}",
                                   name=f"wout{fc}")
                    nc.sync.dma_start(
                        out=t[:], in_=woutT[fc * 128:(fc + 1) * 128, :])
                    wout_sb.append(t)
                nc.sync.dma_start(out=bout_sb[:], in_=boutb[:])

            def load_ctx(b):
                for h in range(HPC):
                    t = const.tile([128, NKC * CW], BF, tag=f"ctx{b % 2}{h}",
                                   name=f"ctx{b}_{h}")
                    nc.sync.dma_start(out=t[:], in_=ctxa[b, h, :, :])
                    ctx_sb[b, h] = t

            def prefetch_x(b):
                """Issue per-(kc, t) xT DMAs and allocate q/k for batch b."""
                qT = qkpool.tile([128, N], BF, tag=f"qT{b % 2}", name=f"qT{b}")
                kT = qkpool.tile([128, N], BF, tag=f"kT{b % 2}", name=f"kT{b}")
                qk_tiles[b] = (qT, kT)
                for t in range(NT):
                    for kc in range(KC):
                        xt = xtpool.tile([128, 512], BF, tag="xt",
                                         name=f"xtb{b}_{kc}_{t}")
                        nc.sync.dma_start(
                            out=xt[:],
                            in_=xT[kc * 128:(kc + 1) * 128,
                                   b * N + t * 512:b * N + (t + 1) * 512])
                        xt_tiles[b, kc, t] = xt

            def qkproj_half(b, t, fb, half):
                """First or second half of one (512-token, q-or-k)
                projection slice — 4 contraction steps each, so a single
                insertion never starves the activation pipeline."""
                qT, kT = qk_tiles[b]
                dst = kT if fb == 1 else qT
                if half == 0:
                    ps = pj_pool.tile([128, 512], F32, tag="pj",
                                      name=f"psq{b}_{t}_{fb}")
                    qkproj_half.ps[b, t, fb] = ps
                ps = qkproj_half.ps[b, t, fb]
                for kc in range(half * 4, half * 4 + 4):
                    nc.tensor.matmul(
                        ps[:], wqk_sb[kc][:, fb * 128:(fb + 1) * 128],
                        xt_tiles[b, kc, t][:],
                        start=(kc == 0), stop=(kc == KC - 1))
                if half == 1:
                    nc.vector.tensor_scalar_add(
                        dst[:, t * 512:(t + 1) * 512], ps[:], bq_sb[fb][:])
            qkproj_half.ps = {}

            def qkproj_unit(b, t, fb):
                qkproj_half(b, t, fb, 0)
                qkproj_half(b, t, fb, 1)

            def outproj_load(b, rc):
                """Stage the re-sharded activations for (b, row-chunk rc).

                For b<3 chunk rc covers out rows b*256+rc*128 (global
                rows c*256+rc*128); for b=3, stripe rc covers global
                rows rc*1024+c*128.
                """
                sls = []
                for fc in range(KC):
                    sl = slpool.tile([128, 128], BF, tag="sl",
                                     name=f"sl{b}_{rc}_{fc}")
                    if b < 3:
                        src = a2a_out[b][fc, :, rc * 128:(rc + 1) * 128]
                    else:
                        src = a2a3_out[rc][fc, :, :]
                    nc.sync.dma_start(out=sl[:], in_=src)
                    sls.append(sl)
                outproj_load.sls[b, rc] = sls
            outproj_load.sls = {}

            def outproj_half(b, rc, n, half):
                """First or second half of 512 output features (n) for
                row-chunk rc of batch b."""
                sls = outproj_load.sls[b, rc]
                if half == 0:
                    pso = pj_pool.tile([128, 512], F32, tag="pj",
                                       name=f"pso{b}_{rc}_{n}")
                    outproj_half.ps[b, rc, n] = pso
                pso = outproj_half.ps[b, rc, n]
                for fc in range(half * 4, half * 4 + 4):
                    nc.tensor.matmul(
                        pso[:], sls[fc][:],
                        wout_sb[fc][:, n * 512:(n + 1) * 512],
                        start=(fc == 0), stop=(fc == KC - 1))
                if half == 1:
                    ob = obpool.tile([128, 512], F32, tag="ob",
                                     name=f"ob{b}_{rc}_{n}")
                    nc.vector.tensor_tensor(
                        out=ob[:], in0=pso[:],
                        in1=bout_sb[:, n * 512:(n + 1) * 512],
                        op=mybir.AluOpType.add)
                    nc.sync.dma_start(
                        out=out[b * RPB + rc * 128:
                                b * RPB + (rc + 1) * 128,
                                n * 512:(n + 1) * 512],
                        in_=ob[:])
            outproj_half.ps = {}

            def outproj_unit(b, rc, n):
                outproj_half(b, rc, n, 0)
                outproj_half(b, rc, n, 1)

            def attention_qg(b, qg, fillers):
                """Both heads' scores+softmax+values for 512 queries.

                fillers: dict kc -> list of thunks emitted after that
                key-chunk's exp (projection work woven into the stream).
                """
                qT, kT = qk_tiles[b]
                q0 = qg * 512
                LAG = 3
                pts = []
                pavs = []

                def chain_step(s):
                    for h in range(HPC):
                        nc.tensor.matmul(
                            pavs[h][:],
                            ctx_sb[b, h][:, s * CW:(s + 1) * CW],
                            pts[s][:, h * 512:(h + 1) * 512],
                            start=(s == 0), stop=(s == NKC - 1))

                for kc in range(NKC):
                    ps = pss_pool.tile([128, 1024], F32, tag="pss",
                                       name=f"pss{b}{qg}{kc}")
                    for h in range(HPC):
                        nc.tensor.matmul(
                            ps[:, h * 512:(h + 1) * 512],
                            kT[h * HD:(h + 1) * HD, kc * 128:(kc + 1) * 128],
                            qT[h * HD:(h + 1) * HD, q0:q0 + 512],
                            start=True, stop=True,
                            tile_position=(h * HD, 0))
                    pt = ptpool.tile([128, 1024], BF, tag="pt",
                                     name=f"pt{b}_{qg}_{kc}")
                    nc.scalar.activation(
                        pt[:], ps[:],
                        mybir.ActivationFunctionType.Exp, scale=SCALE)
                    pts.append(pt)
                    if kc == LAG - 1:
                        for h in range(HPC):
                            pavs.append(pav_pool.tile(
                                [CW, 512], F32, tag="pav",
                                name=f"pav{b}{qg}{h}"))
                    if kc >= LAG:
                        # value-matmul step for an earlier key chunk —
                        # keeps the chains flowing inside the group so
                        # almost nothing drains at the boundary
                        chain_step(kc - LAG)
                    for f in fillers.get(kc, ()):
                        f()
                for s in range(NKC - LAG, NKC):
                    chain_step(s)
                # stage-major normalize: PSUM evictions now (frees pav
                # slots for the next group's chains); the reciprocal and
                # the DRAM-round-trip broadcast (finish_a) and the
                # normalize-multiply + re-shard staging (finish_b) are
                # woven into the NEXT group's stream so the slow
                # reciprocal never gates this pipeline.
                pcs = []
                for h in range(HPC):
                    pc = pcpool.tile([CW, 512], F32, tag="pc",
                                     name=f"pc{b}{qg}{h}")
                    nc.vector.tensor_copy(pc[:], pavs[h][:])
                    pcs.append(pc)

                rb = rbpool.tile([HD, 1024], F32, tag="rb",
                                 name=f"rb{b}{qg}")

                def finish_a(b=b, qg=qg, pcs=pcs, rb=rb):
                    r1 = r1pool.tile([1, 1024], F32, tag="r1",
                                     name=f"r1{b}{qg}")
                    for h in range(HPC):
                        nc.vector.reciprocal(r1[:, h * 512:(h + 1) * 512],
                                             pcs[h][HD:CW, :])
                    scr = rscr[_scr_idx[0] % 8]; _scr_idx[0] += 1
                    nc.sync.dma_start(out=scr[:], in_=r1[:])
                    nc.sync.dma_start(out=rb[:],
                                      in_=scr[:].broadcast_to([HD, 1024]))

                def finish_b(b=b, qg=qg, q0=q0, pcs=pcs, rb=rb):
                    for h in range(HPC):
                        ho = hopool.tile([HD, 512], BF, tag="ho",
                                         name=f"ho{b}{qg}{h}")
                        nc.vector.tensor_tensor(
                            out=ho[:], in0=pcs[h][0:HD, :],
                            in1=rb[:, h * 512:(h + 1) * 512],
                            op=mybir.AluOpType.mult)
                        if b < 3:
                            for half in range(2):
                                j = 2 * qg + half
                                nc.sync.dma_start(
                                    out=a2a_in[b][j, h * HD:(h + 1) * HD, :],
                                    in_=ho[:, half * 256:(half + 1) * 256])
                        else:
                            for m in range(4):
                                row = q0 + m * 128
                                hf, j = row // 1024, (row % 1024) // 128
                                nc.sync.dma_start(
                                    out=a2a3_in[hf][j, h * HD:(h + 1) * HD,
                                                    :],
                                    in_=ho[:, m * 128:(m + 1) * 128])
                return finish_a, finish_b

            def reshard(b):
                nc.gpsimd.collective_compute(
                    "AllToAll", mybir.AluOpType.bypass,
                    replica_groups=[list(range(NC))],
                    ins=[a2a_in[b].ap().opt()],
                    outs=[a2a_out[b].ap().opt()])

            def reshard3(hf):
                nc.gpsimd.collective_compute(
                    "AllToAll", mybir.AluOpType.bypass,
                    replica_groups=[list(range(NC))],
                    ins=[a2a3_in[hf].ap().opt()],
                    outs=[a2a3_out[hf].ap().opt()])

            # ---------------- emission ----------------
            prefetch_x(0)
            load_ctx(0)
            # warm the PE clock while the first x chunks are in flight
            warm_pe(40, "head")
            # scores(qg0, kc) only needs k of token-chunk kc//4 and q of
            # chunk 0 — project those, then start attention immediately
            # and weave the remaining projection slices into qg0/qg1
            qkproj_unit(0, 0, 1)
            qkproj_unit(0, 0, 0)
            prefetch_x(1)
            load_ctx(1)
            load_out_consts()

            pending = []  # deferred normalize-finish of the previous group
            for b in range(B):
                for qg in range(4):
                    fillers = {}
                    if pending:
                        fa, fb_ = pending.pop()
                        fillers.setdefault(2, []).append(fa)
                        fillers.setdefault(12, []).append(fb_)
                    if b == 0:
                        # remaining b0 slices: k(t) must land before
                        # scores reach kc = 4*t; q(t) before group t
                        b0_slots = {0: ((0, 1, 1), (4, 2, 1), (8, 3, 1),
                                        (12, 1, 0)),
                                    1: ((4, 2, 0), (12, 3, 0))}
                        for slot, t, fb in b0_slots.get(qg, ()):
                            fillers.setdefault(slot, []).append(
                                lambda t=t, fb=fb: qkproj_unit(0, t, fb))
                    if b + 1 < B:
                        # q/k projection of the next batch: 8 units per
                        # batch (kept out of b0's warm-up query group)
                        if b == 0:
                            b1_slots = {1: ((3, 0, 1), (10, 0, 0)),
                                        2: ((1, 1, 1), (6, 2, 1),
                                            (10, 1, 0), (14, 2, 0)),
                                        3: ((1, 3, 1), (10, 3, 0))}
                            for slot, t, fb in b1_slots.get(qg, ()):
                                fillers.setdefault(slot, []).append(
                                    lambda t=t, fb=fb: qkproj_unit(1, t, fb))
                        else:
                            t = qg
                            for slot, fb, half in ((1, 1, 0), (3, 1, 1),
                                                   (9, 0, 0), (11, 0, 1)):
                                fillers.setdefault(slot, []).append(
                                    lambda b=b, t=t, fb=fb, half=half:
                                    qkproj_half(b + 1, t, fb, half))
                    if b >= 1 and qg >= 2:
                        # output projection of the previous batch in the
                        # second half of this batch's attention, so its
                        # AllToAll has certainly landed; the loads go
                        # first so the matmuls never wait on DMA
                        rc = qg - 2
                        fillers.setdefault(0, []).append(
                            lambda b=b, rc=rc: outproj_load(b - 1, rc))
                        for slot, n, half in ((4, 0, 0), (6, 0, 1),
                                              (13, 1, 0), (15, 1, 1)):
                            fillers.setdefault(slot, []).append(
                                lambda b=b, rc=rc, n=n, half=half:
                                outproj_half(b - 1, rc, n, half))
                    fins = attention_qg(b, qg, fillers)
                    if qg == 3:
                        # last group of the batch: finish inline so the
                        # batch collective can be emitted right after
                        # (its dependency set must include these DMAs)
                        fins[0]()
                        fins[1]()
                    else:
                        pending.append(fins)
                    if b == 3 and qg == 2:
                        # first-half finishes of b3 (qg0 in qg1, qg1
                        # here) have been emitted by now
                        reshard3(0)
                    if qg == 2 and b + 2 < B:
                        prefetch_x(b + 2)
                        load_ctx(b + 2)
                if b < 3:
                    reshard(b)
                else:
                    reshard3(1)
            # tail: hold the PE warm across the final normalize window,
            # run stripe-0's output projection (its a2a landed after
            # qg2) while the last collective flies, then stripe 1
            warm_pe(80, "tail0")
            outproj_load(3, 0)
            for n in range(2):
                outproj_unit(3, 0, n)
            warm_pe(24, "tail1")
            outproj_load(3, 1)
            for n in range(2):
                outproj_unit(3, 1, n)
    nc.compile()
    return nc


def prep_inputs(x, context, Wqkv, bqkv, Wout, bout):
    """Host-side sharding: returns in_maps for the 8 cores."""
    x = np.asarray(x, np.float32)
    context = np.asarray(context, np.float32)
    Wqkv = np.asarray(Wqkv, np.float32)
    bqkv = np.asarray(bqkv, np.float32)
    Wout = np.asarray(Wout, np.float32)
    bout = np.asarray(bout, np.float32)

    xT = np.ascontiguousarray(x.reshape(BN, DIM).T).astype(BF16)
    woutT = np.ascontiguousarray(Wout.T).astype(BF16)
    boutb = np.broadcast_to(bout, (128, DIM)).astype(np.float32).copy()

    in_maps = []
    for c in range(NC):
        h0 = c * HPC
        # feature order: [q_h0 | q_h1] then [k_h0 | k_h1]
        wq = Wqkv[h0 * HD:(h0 + HPC) * HD]
        wk = Wqkv[DIM + h0 * HD:DIM + (h0 + HPC) * HD]
        wqkT = np.ascontiguousarray(
            np.concatenate([wq, wk], axis=0).T).astype(BF16)
        bq = np.concatenate([bqkv[h0 * HD:(h0 + HPC) * HD],
                             bqkv[DIM + h0 * HD:DIM + (h0 + HPC) * HD]])
        bq = bq.reshape(2 * 128, 1).astype(np.float32)
        ctxa = np.ones((B, HPC, 128, NKC, CW), np.float32)
        for h in range(HPC):
            g = h0 + h
            arr = context[:, :, g * HD:(g + 1) * HD].reshape(B, NKC, 128, HD)
            ctxa[:, h, :, :, :HD] = arr.transpose(0, 2, 1, 3)
        in_maps.append({
            "xT": xT,
            "wqkT": wqkT,
            "bqk": bq,
            "ctxa": ctxa.reshape(B, HPC, 128, NKC * CW).astype(BF16),
            "woutT": woutT,
            "boutb": boutb,
        })
    return in_maps


_NC_CACHE = None


def _get_nc():
    global _NC_CACHE
    if _NC_CACHE is None:
        _NC_CACHE = build()
    return _NC_CACHE


def run(in_maps, trace=False):
    nc = _get_nc()
    res = run_bass_kernel_spmd(nc, in_maps, core_ids=list(range(NC)),
                               trace=trace)
    # core c's out = [B*256, DIM]: batches 0-2 hold rows
    # [c*256:(c+1)*256]; batch 3's two 128-row chunks hold stripes
    # [hf*1024 + c*128 : hf*1024 + (c+1)*128]
    full = np.empty((B, N, DIM), np.float32)
    for c in range(NC):
        o = np.asarray(res.results[c]["out"]).reshape(B, RPB, DIM)
        full[:3, c * RPB:(c + 1) * RPB, :] = o[:3]
        for hf in range(2):
            full[3, hf * 1024 + c * 128:hf * 1024 + (c + 1) * 128, :] = \
                o[3, hf * 128:(hf + 1) * 128]
    return full, res


def kernel(x, context, Wqkv, bqkv, Wout, bout):
    in_maps = prep_inputs(x, context, Wqkv, bqkv, Wout, bout)
    out, _ = run(in_maps, trace=False)
    return out
